# revision 1
# baseline (speedup 1.0000x reference)
"""Causal attention (B=4, T=2048, D=1024) on 8 TRN2 NeuronCores.

Sharding: core c = (batch b = c//2, half h = c%2). Each core computes
attention for 1024 query rows of one batch: 4 slots of 256 rows, with
balanced causal work via block assignment h=0 -> blocks [7,4,3,0],
h=1 -> [6,5,2,1] (blocks of 256 rows). Slot s processes a k-tile
prefix of length CAPS[s] = [16,12,8,4] (k-tiles of 128 keys), which
covers both cores' needs exactly; the causal mask (generated on-device
from qpos/kpos) zeroes any over-computed region. K/V are computed for
the full batch on both cores of a batch (no cross-device comm).

Math per core (all matmul inputs bf16, fp32 PSUM accumulation):
  KT[e,t] = sum_d Wk[d,e] * xT[d,t]        (lhsT=Wk tile, rhs=xT)
  QT[e,q] = sum_d Wq[d,e] * xqT[d,q]
  V[t,e]  = sum_d xT[d,t] * Wv[d,e]        (lhsT=xT tile, rhs=Wv)
  ST[k,q] = sum_e KT[e,k] * QT[e,q]        (lhsT=KT tile, rhs=QT)
  PT[k,q] = exp(ST/sqrt(D)) * (qpos[q] >= kpos[k])   (no max-sub: logits ~N(0,1))
  O[q,e]  = sum_k PT[k,q] * V[k,e];  sum[q] = sum_k PT[k,q] (ones-column matmul)
  out[q,e] = O[q,e] / sum[q]
"""

import numpy as np
import ml_dtypes

import concourse.bacc as bacc
import concourse.bass as bass
import concourse.mybir as mybir
import concourse.tile as tile
from concourse.bass_utils import run_bass_kernel_spmd

BF16 = mybir.dt.bfloat16
F32 = mybir.dt.float32

B, T, D = 4, 2048, 1024
P = 128          # partitions
DT = D // P      # 8 d-tiles
KT_N = T // P    # 16 k-tiles
SLOT_Q = 256
NSLOT = 4
CAPS = [16, 12, 8, 4]           # k-tile prefix length per slot
OFF = [0, 16, 28, 36]           # unit offset per slot (cumsum of CAPS)
NUNIT = sum(CAPS)               # 40
ASSIGN = {0: [7, 4, 3, 0], 1: [6, 5, 2, 1]}   # q-block (of 256) per slot
SCALE = 1.0 / np.sqrt(np.float32(D))

_NC_CACHE = None


def _active(kt):
    """number of slots whose cap exceeds kt (slots are cap-descending)"""
    return sum(1 for c in CAPS if c > kt)


def build_nc():
    nc = bacc.Bacc("TRN2", target_bir_lowering=False, debug=False,
                   enable_asserts=False, enable_partition_id=False)

    xkvT = nc.dram_tensor("xkvT", [D, T], BF16, kind="ExternalInput").ap()
    xqT = nc.dram_tensor("xqT", [D, NSLOT * SLOT_Q], BF16, kind="ExternalInput").ap()
    Wq = nc.dram_tensor("Wq", [D, D], BF16, kind="ExternalInput").ap()
    Wk = nc.dram_tensor("Wk", [D, D], BF16, kind="ExternalInput").ap()
    Wv = nc.dram_tensor("Wv", [D, D], BF16, kind="ExternalInput").ap()
    qpos_d = nc.dram_tensor("qpos", [NSLOT * SLOT_Q], F32, kind="ExternalInput").ap()
    kpos_d = nc.dram_tensor("kpos", [P, KT_N], F32, kind="ExternalInput").ap()
    out_d = nc.dram_tensor("out", [NSLOT * SLOT_Q, D], F32, kind="ExternalOutput").ap()

    NQ = NSLOT * SLOT_Q  # 1024 query rows per core

    with tile.TileContext(nc) as tc:
        with tc.tile_pool(name="sb", bufs=1) as sb, \
             tc.tile_pool(name="ps", bufs=1, space="PSUM") as ps:

            # ---- stage A: load inputs ----
            wq_s = sb.tile([P, DT, D], BF16, tag="w", bufs=3)
            wk_s = sb.tile([P, DT, D], BF16, tag="w", bufs=3)
            wv_s = sb.tile([P, DT, D], BF16, tag="w", bufs=3)
            xkvT_s = sb.tile([P, DT, T], BF16, tag="xkvT", bufs=1)
            xqT_s = sb.tile([P, DT, NQ], BF16, tag="xqT", bufs=1)
            nc.sync.dma_start(out=wk_s, in_=Wk.rearrange("(dt p) e -> p dt e", p=P))
            nc.sync.dma_start(out=xkvT_s, in_=xkvT.rearrange("(dt p) t -> p dt t", p=P))
            nc.sync.dma_start(out=wq_s, in_=Wq.rearrange("(dt p) e -> p dt e", p=P))
            nc.sync.dma_start(out=xqT_s, in_=xqT.rearrange("(dt p) q -> p dt q", p=P))
            nc.sync.dma_start(out=wv_s, in_=Wv.rearrange("(dt p) e -> p dt e", p=P))

            qpos_s = sb.tile([P, NQ], F32, tag="qpos", bufs=1)
            qpos_bcast = bass.AP(tensor=qpos_d.tensor, offset=qpos_d.offset,
                                 ap=[[0, P]] + list(qpos_d.ap))
            nc.gpsimd.dma_start(out=qpos_s, in_=qpos_bcast)
            kpos_s = sb.tile([P, KT_N], F32, tag="kpos", bufs=1)
            nc.sync.dma_start(out=kpos_s, in_=kpos_d)
            ones_s = sb.tile([P, 1], BF16, tag="ones", bufs=1)
            nc.vector.memset(ones_s, 1.0)

            kt_s = sb.tile([P, DT, T], BF16, tag="kt", bufs=1)
            qt_s = sb.tile([P, DT, NQ], BF16, tag="qt", bufs=1)
            v_s = sb.tile([P, KT_N, D], BF16, tag="v", bufs=1)

            # ---- stage B: KT[e,t] ----
            for et in range(DT):
                for tc4 in range(T // 512):
                    pk = ps.tile([P, 1024], F32, tag="big", bufs=3)
                    for dt in range(DT):
                        nc.tensor.matmul(
                            pk[:, 0:512],
                            wk_s[:, dt, et * P:(et + 1) * P],
                            xkvT_s[:, dt, tc4 * 512:(tc4 + 1) * 512],
                            start=(dt == 0), stop=(dt == DT - 1))
                    nc.any.tensor_copy(
                        out=kt_s[:, et, tc4 * 512:(tc4 + 1) * 512], in_=pk[:, 0:512])

            # ---- stage C: QT[e,q] ----
            for et in range(DT):
                for qc in range(NQ // 512):
                    pq = ps.tile([P, 1024], F32, tag="big", bufs=3)
                    for dt in range(DT):
                        nc.tensor.matmul(
                            pq[:, 0:512],
                            wq_s[:, dt, et * P:(et + 1) * P],
                            xqT_s[:, dt, qc * 512:(qc + 1) * 512],
                            start=(dt == 0), stop=(dt == DT - 1))
                    nc.any.tensor_copy(
                        out=qt_s[:, et, qc * 512:(qc + 1) * 512], in_=pq[:, 0:512])

            # ---- stage D: V[t,e] ----
            for tt in range(KT_N):
                for ec in range(D // 512):
                    pv = ps.tile([P, 1024], F32, tag="big", bufs=3)
                    for dt in range(DT):
                        nc.tensor.matmul(
                            pv[:, 0:512],
                            xkvT_s[:, dt, tt * P:(tt + 1) * P],
                            wv_s[:, dt, ec * 512:(ec + 1) * 512],
                            start=(dt == 0), stop=(dt == DT - 1))
                    nc.any.tensor_copy(
                        out=v_s[:, tt, ec * 512:(ec + 1) * 512], in_=pv[:, 0:512])

            # PT units: two halves to fit reused slots of tag "w"
            pt_a = sb.tile([P, 20, SLOT_Q], BF16, tag="w", bufs=3)
            pt_b = sb.tile([P, 20, SLOT_Q], BF16, tag="w", bufs=3)

            def pt_unit(u):
                return pt_a[:, u, :] if u < 20 else pt_b[:, u - 20, :]

            # ---- stage E: ST = KT.T @ QT per k-tile; PT = exp(ST*scale)*mask ----
            for kt in range(KT_N):
                w = SLOT_Q * _active(kt)
                st = ps.tile([P, 1024], F32, tag="big", bufs=3, name=f"st{kt}")
                for dt in range(DT):
                    for p0 in range(0, w, 512):
                        pw = min(512, w - p0)
                        nc.tensor.matmul(
                            st[:, p0:p0 + pw],
                            kt_s[:, dt, kt * P:(kt + 1) * P],
                            qt_s[:, dt, p0:p0 + pw],
                            start=(dt == 0), stop=(dt == DT - 1))
                for s in range(_active(kt)):
                    u = OFF[s] + kt
                    nc.scalar.activation(
                        out=pt_unit(u), in_=st[:, s * SLOT_Q:(s + 1) * SLOT_Q],
                        func=mybir.ActivationFunctionType.Exp, scale=float(SCALE))
                    # mask only where not provably all-keep for both cores
                    min_block = min(ASSIGN[0][s], ASSIGN[1][s])
                    if (kt + 1) * P > min_block * SLOT_Q:
                        m = sb.tile([P, SLOT_Q], BF16, tag="mask", bufs=4,
                                    name=f"m{kt}_{s}")
                        nc.vector.tensor_scalar(
                            out=m, in0=qpos_s[:, s * SLOT_Q:(s + 1) * SLOT_Q],
                            scalar1=kpos_s[:, kt:kt + 1], scalar2=None,
                            op0=mybir.AluOpType.is_ge)
                        nc.vector.tensor_mul(out=pt_unit(u), in0=pt_unit(u), in1=m)

            # ---- stage F: O = PT.T @ [V | 1]; normalize ----
            for s in range(NSLOT):
                for qs in range(SLOT_Q // P):
                    po = ps.tile([P, 1024], F32, tag="big", bufs=3, name=f"po{s}_{qs}")
                    psum = ps.tile([P, 1], F32, tag="sum", bufs=2, name=f"psum{s}_{qs}")
                    for i, kt in enumerate(range(CAPS[s])):
                        lhsT = pt_unit(OFF[s] + kt)[:, qs * P:(qs + 1) * P]
                        fl = dict(start=(i == 0), stop=(i == CAPS[s] - 1))
                        nc.tensor.matmul(po[:, 0:512], lhsT, v_s[:, kt, 0:512], **fl)
                        nc.tensor.matmul(po[:, 512:1024], lhsT, v_s[:, kt, 512:1024], **fl)
                        nc.tensor.matmul(psum, lhsT, ones_s, **fl)
                    recip = sb.tile([P, 1], F32, tag="recip", bufs=4, name=f"r{s}_{qs}")
                    nc.vector.reciprocal(out=recip, in_=psum)
                    o_sb = sb.tile([P, 1024], F32, tag="osb", bufs=2, name=f"o{s}_{qs}")
                    nc.vector.tensor_scalar_mul(out=o_sb, in0=po[:, 0:1024], scalar1=recip)
                    r0 = s * SLOT_Q + qs * P
                    nc.sync.dma_start(out=out_d[r0:r0 + P, :], in_=o_sb)

    nc.compile()
    return nc


def _host_prep(x, Wq, Wk, Wv):
    """Build per-core input maps. x: [B,T,D] fp32."""
    bf = ml_dtypes.bfloat16
    Wq_b, Wk_b, Wv_b = (np.ascontiguousarray(w.astype(bf)) for w in (Wq, Wk, Wv))
    kpos = (np.arange(T, dtype=np.float32).reshape(KT_N, P).T).copy()  # [P, KT_N]
    in_maps = []
    for c in range(8):
        b, h = divmod(c, 2)
        blocks = ASSIGN[h]
        xb = x[b].astype(bf)                      # [T, D]
        xkvT = np.ascontiguousarray(xb.T)         # [D, T]
        xq = np.concatenate([xb[g * SLOT_Q:(g + 1) * SLOT_Q] for g in blocks], axis=0)
        xqT = np.ascontiguousarray(xq.T)          # [D, 1024]
        qpos = np.concatenate([
            np.arange(g * SLOT_Q, (g + 1) * SLOT_Q, dtype=np.float32) for g in blocks])
        in_maps.append({
            "xkvT": xkvT, "xqT": xqT,
            "Wq": Wq_b, "Wk": Wk_b, "Wv": Wv_b,
            "qpos": qpos, "kpos": kpos,
        })
    return in_maps


def _reassemble(results, dtype=np.float32):
    out = np.empty((B, T, D), dtype=dtype)
    for c in range(8):
        b, h = divmod(c, 2)
        o = results[c]["out"]                     # [1024, D]
        for s, g in enumerate(ASSIGN[h]):
            out[b, g * SLOT_Q:(g + 1) * SLOT_Q] = o[s * SLOT_Q:(s + 1) * SLOT_Q]
    return out


def kernel(**inputs):
    global _NC_CACHE
    x = np.asarray(inputs["x"], dtype=np.float32)
    Wq = np.asarray(inputs["Wq"], dtype=np.float32)
    Wk = np.asarray(inputs["Wk"], dtype=np.float32)
    Wv = np.asarray(inputs["Wv"], dtype=np.float32)
    if _NC_CACHE is None:
        _NC_CACHE = build_nc()
    nc = _NC_CACHE
    in_maps = _host_prep(x, Wq, Wk, Wv)
    res = run_bass_kernel_spmd(nc, in_maps, core_ids=list(range(8)))
    return _reassemble(res.results)


if __name__ == "__main__":
    rng = np.random.default_rng(0)
    x = rng.standard_normal((B, T, D), dtype=np.float32)
    Wq = rng.standard_normal((D, D), dtype=np.float32) / np.sqrt(D)
    Wk = rng.standard_normal((D, D), dtype=np.float32) / np.sqrt(D)
    Wv = rng.standard_normal((D, D), dtype=np.float32) / np.sqrt(D)
    out = kernel(x=x, Wq=Wq, Wk=Wk, Wv=Wv)
    print("out", out.shape, out.dtype, np.abs(out).max())
